# revision 22
# baseline (speedup 1.0000x reference)
"""MoE feed-forward (top-2 routing, capacity dropping) on 8 Trainium2 cores.

Strategy (expert-parallel, per sharding hint):
  - Host (numpy): gating matmul, top-2 + softmax, capacity-based dropping,
    renormalization, dispatch-buffer build (the "all-to-all" — each core's
    slice is assembled host-side since inputs/outputs are full host arrays).
  - Device (SPMD over 8 cores, core e == expert e): the grouped FFN
        h   = relu(X_e @ W1_e^T)      (capacity x Dff)
        out = h @ W2_e^T              (capacity x D)
    which is ~99.9% of the FLOPs. Matmul inputs are bf16 (full PE rate,
    FWL weight loads); accumulation is fp32 in PSUM, output fp32.
  - Host: combine (gather slot outputs back to tokens) + aux loss.

All shapes are hardcoded for the fixed problem instance:
  B,S,D,Dff,E,k = 4,2048,1024,4096,8,2 ; capacity = ceil(1.25*B*S/E) = 1280
"""

import math
import os
from contextlib import ExitStack

import numpy as np

B, S, D, DFF, E, TOPK = 4, 2048, 1024, 4096, 8, 2
N = B * S                                    # 8192 tokens
NK = N * TOPK                                # 16384 slots
CAP = int(math.ceil(1.25 * N / E))           # 1280
P = 128
KO = D // P                                  # 8 k-tiles (contraction d)
FO = DFF // P                                # 32 f-tiles
CSUBS = [(0, 512), (512, 1024), (1024, 1280)]  # capacity split; one PSUM bank

# "bf16" (both matmul operands bf16) measured fastest at rel err 3.4e-3;
# "f32r" keeps full-precision inputs at ~9% more HW time (rel err 2.4e-3).
VARIANT = os.environ.get("MOE_VARIANT", "bf16")
_SESSION = None          # built+compiled bass module, reused across calls
LAST_RESULTS = None      # BassKernelResults of the most recent device run


def _build_ffn_bass(variant="bf16w"):
    """variant: "f32r" (x,W1 f32r), "bf16w" (W1 bf16, x f32r), "bf16" (both)."""
    import concourse.bass as bass  # noqa: F401
    import concourse.tile as tile
    from concourse import bacc, mybir
    from concourse.vector_clock import ScopedClock

    class _FastTC(tile.TileContext):
        """TileContext with a lighter kernel epilogue: keep the drain (it
        carries the completion waits for the output DMAs) and one all-engine
        barrier, but skip the semaphore clear + second barrier — nothing runs
        after this one-shot kernel, so resetting semaphores is dead time."""

        def _drain_and_barrier(self, tick_clock, wait_clock):
            drain_inst = self.nc.sync.drain()
            wait_clock.add_sem_waits(
                drain_inst.ins, ScopedClock({None: tick_clock.global_clock})
            )
            self.nc.all_engine_barrier()
            popped = self.nc._tile_sem_poison_stack.pop()
            assert popped is self._sem_poison

    f32 = mybir.dt.float32
    f32r = mybir.dt.float32r
    bf16 = mybir.dt.bfloat16
    dt_w1 = f32r if variant == "f32r" else bf16
    dt_x = bf16 if variant == "bf16" else f32r
    relu = mybir.ActivationFunctionType.Relu

    nc = bacc.Bacc(
        "TRN2",
        target_bir_lowering=False,
        debug=False,
        enable_asserts=False,
        num_devices=E,
    )

    # Host pre-tiles the operands so every DMA lands contiguous:
    #   xt{s} [ko, ki, c]     = X_e^T tiles, one tensor per c-subblock
    #   w1 [fb, ki, ko*128+f] = W1_e^T tiles
    #   w2 [do, fi, fo*128+d] = W2_e^T tiles
    #   yt [do, ki, c]        = out^T tiles          (f32)
    xt_ds = [
        nc.dram_tensor(f"xt{s}", [KO, P, c1 - c0], dt_x,
                       kind="ExternalInput").ap()
        for s, (c0, c1) in enumerate(CSUBS)
    ]
    w1_d = nc.dram_tensor("w1", [FO, P, KO * P], dt_w1, kind="ExternalInput").ap()
    w2_d = nc.dram_tensor("w2", [KO, P, FO * P], bf16, kind="ExternalInput").ap()
    yt_d = nc.dram_tensor("yt", [KO, P, CAP], f32, kind="ExternalOutput").ap()

    with _FastTC(nc) as tc, ExitStack() as ctx:
        xpool = ctx.enter_context(tc.tile_pool(name="xpool", bufs=1))
        w1pool = ctx.enter_context(tc.tile_pool(name="w1pool", bufs=4))
        w2pool = ctx.enter_context(tc.tile_pool(name="w2pool", bufs=3))
        hpool = ctx.enter_context(tc.tile_pool(name="hpool", bufs=1))
        opool = ctx.enter_context(tc.tile_pool(name="opool", bufs=4))
        cpool = ctx.enter_context(tc.tile_pool(name="cpool", bufs=1))
        # PSUM budget: 8 banks of (128 x 2KB). One pool shared by both
        # phases (and the HAM warmup), one tag per c-subblock, 3/3/2 slots
        # -> 8 banks total.
        pspool = ctx.enter_context(tc.tile_pool(name="ps", bufs=1, space="PSUM"))
        PS_BUFS = [3, 3, 2]

        bias = cpool.tile([P, 1], f32, name="bias")
        nc.any.memset(bias[:], 0.0)

        # HAM warmup: the PE clock-gate only opens (1.2 -> 2.4 GHz) after
        # ~3.4us of sustained matmul activity, and the DMA-paced first real
        # matmuls are too sparse to trip it (measured: warm at 28us without
        # this). Burn the input-DMA wait on dummy matmuls over a zeroed tile
        # so the real stream starts at full clock.
        warm = cpool.tile([P, 512], bf16, name="warm")
        nc.any.memset(warm[:], 0.0)
        wps = pspool.tile([P, 512], f32, name="ps_2", bufs=2)
        for _ in range(8):
            nc.tensor.matmul(wps[:], warm[:, :P], warm[:], start=True,
                             stop=True)

        # Startup order matters: the PE can start fb=0/s=0 after just
        # w1[0] + the s=0 third of xt. Interleave the first w1 prefetches
        # with the three contiguous xt slabs so neither stream starves.
        w1_pref = {}
        xts = [xpool.tile([P, CAP], dt_x, name=f"xt{ko}") for ko in range(KO)]
        for s, (c0, c1) in enumerate(CSUBS):
            if s < w1pool.bufs:
                t = w1pool.tile([P, KO * P], dt_w1, name="w1t")
                if s == 0:
                    # head first: fb=0's first chain needs only w1[0][ko=0]
                    # and xt[0][s=0]
                    nc.sync.dma_start(t[:, :P], w1_d[0][:, :P])
                    nc.sync.dma_start(xts[0][:, c0:c1], xt_ds[0][0])
                    nc.sync.dma_start(t[:, P:], w1_d[0][:, P:])
                else:
                    nc.sync.dma_start(t[:], w1_d[s])
                w1_pref[s] = t
            for ko in range(KO):
                if s == 0 and ko == 0:
                    continue
                nc.sync.dma_start(xts[ko][:, c0:c1], xt_ds[s][ko])
        t = w1pool.tile([P, KO * P], dt_w1, name="w1t")
        nc.sync.dma_start(t[:], w1_d[len(CSUBS)])
        w1_pref[len(CSUBS)] = t

        # h holds relu(W1 X^T) in f-major layout: 32 f-tiles side by side,
        # each (128, CAP), bf16 -> 80 KB/partition.
        h = hpool.tile([P, FO * CAP], bf16, name="h")

        # ---- phase 1: h[f, c] = relu(sum_d W1t[d, f] * Xt[d, c]) ----
        for fb in range(FO):
            if fb in w1_pref:
                w1t = w1_pref.pop(fb)
            else:
                w1t = w1pool.tile([P, KO * P], dt_w1, name="w1t")
                nc.sync.dma_start(w1t[:], w1_d[fb])
            for s, (c0, c1) in enumerate(CSUBS):
                ps = pspool.tile([P, 512], f32, name=f"ps_{s}",
                                 bufs=PS_BUFS[s])[:, : c1 - c0]
                for ko in range(KO):
                    nc.tensor.matmul(
                        ps[:],
                        w1t[:, ko * P:(ko + 1) * P],
                        xts[ko][:, c0:c1],
                        start=(ko == 0),
                        stop=(ko == KO - 1),
                    )
                nc.scalar.activation(
                    h[:, fb * CAP + c0:fb * CAP + c1], ps[:], relu,
                    bias=bias[:],
                )

        # ---- phase 2: y[d, c] = sum_f W2t[f, d] * h[f, c], bf16 ----
        for do in range(KO):
            w2t = w2pool.tile([P, FO * P], bf16, name="w2t")
            nc.sync.dma_start(w2t[:], w2_d[do])
            for s, (c0, c1) in enumerate(CSUBS):
                ps = pspool.tile([P, 512], f32, name=f"ps_{s}",
                                 bufs=PS_BUFS[s])[:, : c1 - c0]
                for fo in range(FO):
                    nc.tensor.matmul(
                        ps[:],
                        w2t[:, fo * P:(fo + 1) * P],
                        h[:, fo * CAP + c0:fo * CAP + c1],
                        start=(fo == 0),
                        stop=(fo == FO - 1),
                    )
                ot = opool.tile([P, 512], f32, name="ot")[:, : c1 - c0]
                nc.vector.tensor_copy(out=ot[:], in_=ps[:])
                nc.sync.dma_start(yt_d[do][:, c0:c1], ot[:])

    nc.compile()
    return nc


def _get_session():
    global _SESSION
    if _SESSION is None:
        _SESSION = _build_ffn_bass(VARIANT)
    return _SESSION


def _route(x_flat: np.ndarray, Wg: np.ndarray):
    """Mirror of the reference gating: top-2, softmax, capacity dropping in
    flat slot order, renormalize. Returns per-slot expert, position, weight."""
    logits = x_flat @ Wg.T.astype(np.float32)                 # (N, E) f32
    order = np.argsort(-logits, axis=1, kind="stable")[:, :TOPK]
    scores = np.take_along_axis(logits, order, axis=1)        # (N, k)
    m = scores.max(axis=1, keepdims=True)
    ex = np.exp(scores - m, dtype=np.float32)
    probs = ex / ex.sum(axis=1, keepdims=True)

    flat_idx = order.reshape(-1)                              # (NK,)
    onehot = flat_idx[:, None] == np.arange(E)[None, :]
    position = np.cumsum(onehot, axis=0)[np.arange(NK), flat_idx] - 1
    keep = position < CAP

    pk = probs.reshape(-1) * keep
    pk = pk.reshape(N, TOPK)
    pk = pk / (pk.sum(axis=1, keepdims=True) + np.float32(1e-9))
    flat_w = pk.reshape(-1).astype(np.float32)
    valid = flat_w > 0
    return flat_idx, position.astype(np.int64), flat_w, valid


def kernel(x, Wg, W1, W2, k):
    global LAST_RESULTS
    import ml_dtypes

    from concourse import bass_utils

    assert int(k) == TOPK
    x = np.asarray(x, np.float32)
    Wg = np.asarray(Wg, np.float32)
    W1 = np.asarray(W1, np.float32)
    W2 = np.asarray(W2, np.float32)

    x_flat = x.reshape(N, D)
    flat_idx, position, flat_w, valid = _route(x_flat, Wg)
    token_ids = np.repeat(np.arange(N), TOPK)

    # Dispatch: X_all[e, pos] = x[token] * w for each valid slot (positions
    # are unique per expert by construction, so plain assignment works).
    X_all = np.zeros((E, CAP, D), np.float32)
    v = np.nonzero(valid)[0]
    X_all[flat_idx[v], position[v]] = x_flat[token_ids[v]] * flat_w[v, None]

    nc = _get_session()

    in_maps = []
    for e in range(E):
        xt = np.ascontiguousarray(X_all[e].T).reshape(KO, P, CAP)
        w1 = (
            W1[e].T.reshape(KO, P, FO, P).transpose(2, 1, 0, 3)
            .reshape(FO, P, KO * P)
        )
        w2 = (
            W2[e].T.reshape(FO, P, KO, P).transpose(2, 1, 0, 3)
            .reshape(KO, P, FO * P)
        )
        w1_np = np.ascontiguousarray(w1)
        if VARIANT in ("bf16", "bf16w"):
            w1_np = w1_np.astype(ml_dtypes.bfloat16)
        m = {
            "w1": w1_np,
            "w2": np.ascontiguousarray(w2).astype(ml_dtypes.bfloat16),
        }
        for s, (c0, c1) in enumerate(CSUBS):
            xs = np.ascontiguousarray(xt[:, :, c0:c1])
            if VARIANT == "bf16":
                xs = xs.astype(ml_dtypes.bfloat16)
            m[f"xt{s}"] = xs
        in_maps.append(m)

    try:
        res = bass_utils.run_bass_kernel_spmd(
            nc, in_maps, core_ids=list(range(E))
        )
    except ModuleNotFoundError:
        # BASS_TRACE was set but this image lacks the axon NTFF hook module;
        # rerun without tracing.
        os.environ["BASS_NEVER_TRACE"] = "1"
        res = bass_utils.run_bass_kernel_spmd(
            nc, in_maps, core_ids=list(range(E))
        )
    LAST_RESULTS = res

    out_all = np.empty((E, CAP, D), np.float32)
    for e in range(E):
        out_all[e] = res.results[e]["yt"].reshape(D, CAP).T

    # Combine: each token sums its (up to k) slot outputs; weight was already
    # applied pre-dispatch.
    y_flat = np.zeros((N, D), np.float32)
    pos2 = position.reshape(N, TOPK)
    idx2 = flat_idx.reshape(N, TOPK)
    val2 = valid.reshape(N, TOPK)
    for j in range(TOPK):
        contrib = out_all[idx2[:, j], np.clip(pos2[:, j], 0, CAP - 1)]
        y_flat += contrib * val2[:, j, None].astype(np.float32)
    y = y_flat.reshape(B, S, D)

    # Load-balance aux loss.
    token_ctr = np.bincount(
        flat_idx, weights=valid.astype(np.float64), minlength=E
    ).astype(np.float32)
    imp_ctr = np.bincount(
        flat_idx, weights=flat_w.astype(np.float64), minlength=E
    ).astype(np.float32)
    aux = np.float32(
        np.sum((token_ctr / token_ctr.sum()) * (imp_ctr / imp_ctr.sum())) * E
    )
    return y, np.asarray(aux, np.float32)


# revision 23
# speedup vs baseline: 1.0069x; 1.0069x over previous
"""MoE feed-forward (top-2 routing, capacity dropping) on 8 Trainium2 cores.

Strategy (expert-parallel, per sharding hint):
  - Host (numpy): gating matmul, top-2 + softmax, capacity-based dropping,
    renormalization, dispatch-buffer build (the "all-to-all" — each core's
    slice is assembled host-side since inputs/outputs are full host arrays).
  - Device (SPMD over 8 cores, core e == expert e): the grouped FFN
        h   = relu(X_e @ W1_e^T)      (capacity x Dff)
        out = h @ W2_e^T              (capacity x D)
    which is ~99.9% of the FLOPs. Matmul inputs are bf16 (full PE rate,
    FWL weight loads); accumulation is fp32 in PSUM, output fp32.
  - Host: combine (gather slot outputs back to tokens) + aux loss.

All shapes are hardcoded for the fixed problem instance:
  B,S,D,Dff,E,k = 4,2048,1024,4096,8,2 ; capacity = ceil(1.25*B*S/E) = 1280
"""

import math
import os
from contextlib import ExitStack

import numpy as np

B, S, D, DFF, E, TOPK = 4, 2048, 1024, 4096, 8, 2
N = B * S                                    # 8192 tokens
NK = N * TOPK                                # 16384 slots
CAP = int(math.ceil(1.25 * N / E))           # 1280
P = 128
KO = D // P                                  # 8 k-tiles (contraction d)
FO = DFF // P                                # 32 f-tiles
CSUBS = [(0, 512), (512, 1024), (1024, 1280)]  # capacity split; one PSUM bank

# "bf16" (both matmul operands bf16) measured fastest at rel err 3.4e-3;
# "f32r" keeps full-precision inputs at ~9% more HW time (rel err 2.4e-3).
VARIANT = os.environ.get("MOE_VARIANT", "bf16")
_SESSION = None          # built+compiled bass module, reused across calls
LAST_RESULTS = None      # BassKernelResults of the most recent device run


def _build_ffn_bass(variant="bf16w"):
    """variant: "f32r" (x,W1 f32r), "bf16w" (W1 bf16, x f32r), "bf16" (both)."""
    import concourse.bass as bass  # noqa: F401
    import concourse.tile as tile
    from concourse import bacc, mybir
    from concourse.vector_clock import ScopedClock

    class _FastTC(tile.TileContext):
        """TileContext with a lighter kernel epilogue: keep the drain (it
        carries the completion waits for the output DMAs) and one all-engine
        barrier, but skip the semaphore clear + second barrier — nothing runs
        after this one-shot kernel, so resetting semaphores is dead time."""

        def _drain_and_barrier(self, tick_clock, wait_clock):
            drain_inst = self.nc.sync.drain()
            wait_clock.add_sem_waits(
                drain_inst.ins, ScopedClock({None: tick_clock.global_clock})
            )
            popped = self.nc._tile_sem_poison_stack.pop()
            assert popped is self._sem_poison

    f32 = mybir.dt.float32
    f32r = mybir.dt.float32r
    bf16 = mybir.dt.bfloat16
    dt_w1 = f32r if variant == "f32r" else bf16
    dt_x = bf16 if variant == "bf16" else f32r
    relu = mybir.ActivationFunctionType.Relu

    nc = bacc.Bacc(
        "TRN2",
        target_bir_lowering=False,
        debug=False,
        enable_asserts=False,
        num_devices=E,
    )

    # Host pre-tiles the operands so every DMA lands contiguous:
    #   xt{s} [ko, ki, c]     = X_e^T tiles, one tensor per c-subblock
    #   w1 [fb, ki, ko*128+f] = W1_e^T tiles
    #   w2 [do, fi, fo*128+d] = W2_e^T tiles
    #   yt [do, ki, c]        = out^T tiles          (f32)
    xt_ds = [
        nc.dram_tensor(f"xt{s}", [KO, P, c1 - c0], dt_x,
                       kind="ExternalInput").ap()
        for s, (c0, c1) in enumerate(CSUBS)
    ]
    w1_d = nc.dram_tensor("w1", [FO, P, KO * P], dt_w1, kind="ExternalInput").ap()
    w2_d = nc.dram_tensor("w2", [KO, P, FO * P], bf16, kind="ExternalInput").ap()
    yt_d = nc.dram_tensor("yt", [KO, P, CAP], f32, kind="ExternalOutput").ap()

    with _FastTC(nc) as tc, ExitStack() as ctx:
        xpool = ctx.enter_context(tc.tile_pool(name="xpool", bufs=1))
        w1pool = ctx.enter_context(tc.tile_pool(name="w1pool", bufs=4))
        w2pool = ctx.enter_context(tc.tile_pool(name="w2pool", bufs=3))
        hpool = ctx.enter_context(tc.tile_pool(name="hpool", bufs=1))
        opool = ctx.enter_context(tc.tile_pool(name="opool", bufs=4))
        cpool = ctx.enter_context(tc.tile_pool(name="cpool", bufs=1))
        # PSUM budget: 8 banks of (128 x 2KB). One pool shared by both
        # phases (and the HAM warmup), one tag per c-subblock, 3/3/2 slots
        # -> 8 banks total.
        pspool = ctx.enter_context(tc.tile_pool(name="ps", bufs=1, space="PSUM"))
        PS_BUFS = [3, 3, 2]

        bias = cpool.tile([P, 1], f32, name="bias")
        nc.any.memset(bias[:], 0.0)

        # HAM warmup: the PE clock-gate only opens (1.2 -> 2.4 GHz) after
        # ~3.4us of sustained matmul activity, and the DMA-paced first real
        # matmuls are too sparse to trip it (measured: warm at 28us without
        # this). Burn the input-DMA wait on dummy matmuls over a zeroed tile
        # so the real stream starts at full clock.
        warm = cpool.tile([P, 512], bf16, name="warm")
        nc.any.memset(warm[:], 0.0)
        wps = pspool.tile([P, 512], f32, name="ps_2", bufs=2)
        for _ in range(8):
            nc.tensor.matmul(wps[:], warm[:, :P], warm[:], start=True,
                             stop=True)

        # Startup order matters: the PE can start fb=0/s=0 after just
        # w1[0] + the s=0 third of xt. Interleave the first w1 prefetches
        # with the three contiguous xt slabs so neither stream starves.
        w1_pref = {}
        xts = [xpool.tile([P, CAP], dt_x, name=f"xt{ko}") for ko in range(KO)]
        for s, (c0, c1) in enumerate(CSUBS):
            if s < w1pool.bufs:
                t = w1pool.tile([P, KO * P], dt_w1, name="w1t")
                if s == 0:
                    # head first: fb=0's first chain needs only w1[0][ko=0]
                    # and xt[0][s=0]
                    nc.sync.dma_start(t[:, :P], w1_d[0][:, :P])
                    nc.sync.dma_start(xts[0][:, c0:c1], xt_ds[0][0])
                    nc.sync.dma_start(t[:, P:], w1_d[0][:, P:])
                else:
                    nc.sync.dma_start(t[:], w1_d[s])
                w1_pref[s] = t
            for ko in range(KO):
                if s == 0 and ko == 0:
                    continue
                nc.sync.dma_start(xts[ko][:, c0:c1], xt_ds[s][ko])
        t = w1pool.tile([P, KO * P], dt_w1, name="w1t")
        nc.sync.dma_start(t[:], w1_d[len(CSUBS)])
        w1_pref[len(CSUBS)] = t

        # h holds relu(W1 X^T) in f-major layout: 32 f-tiles side by side,
        # each (128, CAP), bf16 -> 80 KB/partition.
        h = hpool.tile([P, FO * CAP], bf16, name="h")

        # ---- phase 1: h[f, c] = relu(sum_d W1t[d, f] * Xt[d, c]) ----
        for fb in range(FO):
            if fb in w1_pref:
                w1t = w1_pref.pop(fb)
            else:
                w1t = w1pool.tile([P, KO * P], dt_w1, name="w1t")
                nc.sync.dma_start(w1t[:], w1_d[fb])
            for s, (c0, c1) in enumerate(CSUBS):
                ps = pspool.tile([P, 512], f32, name=f"ps_{s}",
                                 bufs=PS_BUFS[s])[:, : c1 - c0]
                for ko in range(KO):
                    nc.tensor.matmul(
                        ps[:],
                        w1t[:, ko * P:(ko + 1) * P],
                        xts[ko][:, c0:c1],
                        start=(ko == 0),
                        stop=(ko == KO - 1),
                    )
                nc.scalar.activation(
                    h[:, fb * CAP + c0:fb * CAP + c1], ps[:], relu,
                    bias=bias[:],
                )

        # ---- phase 2: y[d, c] = sum_f W2t[f, d] * h[f, c], bf16 ----
        for do in range(KO):
            w2t = w2pool.tile([P, FO * P], bf16, name="w2t")
            nc.sync.dma_start(w2t[:], w2_d[do])
            for s, (c0, c1) in enumerate(CSUBS):
                ps = pspool.tile([P, 512], f32, name=f"ps_{s}",
                                 bufs=PS_BUFS[s])[:, : c1 - c0]
                for fo in range(FO):
                    nc.tensor.matmul(
                        ps[:],
                        w2t[:, fo * P:(fo + 1) * P],
                        h[:, fo * CAP + c0:fo * CAP + c1],
                        start=(fo == 0),
                        stop=(fo == FO - 1),
                    )
                ot = opool.tile([P, 512], f32, name="ot")[:, : c1 - c0]
                nc.vector.tensor_copy(out=ot[:], in_=ps[:])
                nc.sync.dma_start(yt_d[do][:, c0:c1], ot[:])

    nc.compile()
    return nc


def _get_session():
    global _SESSION
    if _SESSION is None:
        _SESSION = _build_ffn_bass(VARIANT)
    return _SESSION


def _route(x_flat: np.ndarray, Wg: np.ndarray):
    """Mirror of the reference gating: top-2, softmax, capacity dropping in
    flat slot order, renormalize. Returns per-slot expert, position, weight."""
    logits = x_flat @ Wg.T.astype(np.float32)                 # (N, E) f32
    order = np.argsort(-logits, axis=1, kind="stable")[:, :TOPK]
    scores = np.take_along_axis(logits, order, axis=1)        # (N, k)
    m = scores.max(axis=1, keepdims=True)
    ex = np.exp(scores - m, dtype=np.float32)
    probs = ex / ex.sum(axis=1, keepdims=True)

    flat_idx = order.reshape(-1)                              # (NK,)
    onehot = flat_idx[:, None] == np.arange(E)[None, :]
    position = np.cumsum(onehot, axis=0)[np.arange(NK), flat_idx] - 1
    keep = position < CAP

    pk = probs.reshape(-1) * keep
    pk = pk.reshape(N, TOPK)
    pk = pk / (pk.sum(axis=1, keepdims=True) + np.float32(1e-9))
    flat_w = pk.reshape(-1).astype(np.float32)
    valid = flat_w > 0
    return flat_idx, position.astype(np.int64), flat_w, valid


def kernel(x, Wg, W1, W2, k):
    global LAST_RESULTS
    import ml_dtypes

    from concourse import bass_utils

    assert int(k) == TOPK
    x = np.asarray(x, np.float32)
    Wg = np.asarray(Wg, np.float32)
    W1 = np.asarray(W1, np.float32)
    W2 = np.asarray(W2, np.float32)

    x_flat = x.reshape(N, D)
    flat_idx, position, flat_w, valid = _route(x_flat, Wg)
    token_ids = np.repeat(np.arange(N), TOPK)

    # Dispatch: X_all[e, pos] = x[token] * w for each valid slot (positions
    # are unique per expert by construction, so plain assignment works).
    X_all = np.zeros((E, CAP, D), np.float32)
    v = np.nonzero(valid)[0]
    X_all[flat_idx[v], position[v]] = x_flat[token_ids[v]] * flat_w[v, None]

    nc = _get_session()

    in_maps = []
    for e in range(E):
        xt = np.ascontiguousarray(X_all[e].T).reshape(KO, P, CAP)
        w1 = (
            W1[e].T.reshape(KO, P, FO, P).transpose(2, 1, 0, 3)
            .reshape(FO, P, KO * P)
        )
        w2 = (
            W2[e].T.reshape(FO, P, KO, P).transpose(2, 1, 0, 3)
            .reshape(KO, P, FO * P)
        )
        w1_np = np.ascontiguousarray(w1)
        if VARIANT in ("bf16", "bf16w"):
            w1_np = w1_np.astype(ml_dtypes.bfloat16)
        m = {
            "w1": w1_np,
            "w2": np.ascontiguousarray(w2).astype(ml_dtypes.bfloat16),
        }
        for s, (c0, c1) in enumerate(CSUBS):
            xs = np.ascontiguousarray(xt[:, :, c0:c1])
            if VARIANT == "bf16":
                xs = xs.astype(ml_dtypes.bfloat16)
            m[f"xt{s}"] = xs
        in_maps.append(m)

    try:
        res = bass_utils.run_bass_kernel_spmd(
            nc, in_maps, core_ids=list(range(E))
        )
    except ModuleNotFoundError:
        # BASS_TRACE was set but this image lacks the axon NTFF hook module;
        # rerun without tracing.
        os.environ["BASS_NEVER_TRACE"] = "1"
        res = bass_utils.run_bass_kernel_spmd(
            nc, in_maps, core_ids=list(range(E))
        )
    LAST_RESULTS = res

    out_all = np.empty((E, CAP, D), np.float32)
    for e in range(E):
        out_all[e] = res.results[e]["yt"].reshape(D, CAP).T

    # Combine: each token sums its (up to k) slot outputs; weight was already
    # applied pre-dispatch.
    y_flat = np.zeros((N, D), np.float32)
    pos2 = position.reshape(N, TOPK)
    idx2 = flat_idx.reshape(N, TOPK)
    val2 = valid.reshape(N, TOPK)
    for j in range(TOPK):
        contrib = out_all[idx2[:, j], np.clip(pos2[:, j], 0, CAP - 1)]
        y_flat += contrib * val2[:, j, None].astype(np.float32)
    y = y_flat.reshape(B, S, D)

    # Load-balance aux loss.
    token_ctr = np.bincount(
        flat_idx, weights=valid.astype(np.float64), minlength=E
    ).astype(np.float32)
    imp_ctr = np.bincount(
        flat_idx, weights=flat_w.astype(np.float64), minlength=E
    ).astype(np.float32)
    aux = np.float32(
        np.sum((token_ctr / token_ctr.sum()) * (imp_ctr / imp_ctr.sum())) * E
    )
    return y, np.asarray(aux, np.float32)


# revision 26
# speedup vs baseline: 1.0083x; 1.0013x over previous
"""MoE feed-forward (top-2 routing, capacity dropping) on 8 Trainium2 cores.

Strategy (expert-parallel, per sharding hint):
  - Host (numpy): gating matmul, top-2 + softmax, capacity-based dropping,
    renormalization, dispatch-buffer build (the "all-to-all" — each core's
    slice is assembled host-side since inputs/outputs are full host arrays).
  - Device (SPMD over 8 cores, core e == expert e): the grouped FFN
        h   = relu(X_e @ W1_e^T)      (capacity x Dff)
        out = h @ W2_e^T              (capacity x D)
    which is ~99.9% of the FLOPs. Matmul inputs are bf16 (full PE rate,
    FWL weight loads); accumulation is fp32 in PSUM, output fp32.
  - Host: combine (gather slot outputs back to tokens) + aux loss.

All shapes are hardcoded for the fixed problem instance:
  B,S,D,Dff,E,k = 4,2048,1024,4096,8,2 ; capacity = ceil(1.25*B*S/E) = 1280
"""

import math
import os
from contextlib import ExitStack

import numpy as np

B, S, D, DFF, E, TOPK = 4, 2048, 1024, 4096, 8, 2
N = B * S                                    # 8192 tokens
NK = N * TOPK                                # 16384 slots
CAP = int(math.ceil(1.25 * N / E))           # 1280
P = 128
KO = D // P                                  # 8 k-tiles (contraction d)
FO = DFF // P                                # 32 f-tiles
# capacity split per chain: matmul PSUM writes are capped at one bank
# (512 f32) regardless of moving dtype (ISA check rejects wider).
CSUBS = [(0, 512), (512, 1024), (1024, 1280)]

# "bf16" (both matmul operands bf16) measured fastest at rel err 3.4e-3;
# "f32r" keeps full-precision inputs at ~9% more HW time (rel err 2.4e-3).
VARIANT = os.environ.get("MOE_VARIANT", "bf16")
_SESSION = None          # built+compiled bass module, reused across calls
LAST_RESULTS = None      # BassKernelResults of the most recent device run


def _build_ffn_bass(variant="bf16w"):
    """variant: "f32r" (x,W1 f32r), "bf16w" (W1 bf16, x f32r), "bf16" (both)."""
    import concourse.bass as bass  # noqa: F401
    import concourse.tile as tile
    from concourse import bacc, mybir
    from concourse.vector_clock import ScopedClock

    class _FastTC(tile.TileContext):
        """TileContext with a lighter kernel epilogue: keep the drain (it
        carries the completion waits for the output DMAs) and one all-engine
        barrier, but skip the semaphore clear + second barrier — nothing runs
        after this one-shot kernel, so resetting semaphores is dead time."""

        def _drain_and_barrier(self, tick_clock, wait_clock):
            drain_inst = self.nc.sync.drain()
            wait_clock.add_sem_waits(
                drain_inst.ins, ScopedClock({None: tick_clock.global_clock})
            )
            popped = self.nc._tile_sem_poison_stack.pop()
            assert popped is self._sem_poison

    f32 = mybir.dt.float32
    f32r = mybir.dt.float32r
    bf16 = mybir.dt.bfloat16
    dt_w1 = f32r if variant == "f32r" else bf16
    dt_x = bf16 if variant == "bf16" else f32r
    relu = mybir.ActivationFunctionType.Relu

    nc = bacc.Bacc(
        "TRN2",
        target_bir_lowering=False,
        debug=False,
        enable_asserts=False,
        num_devices=E,
    )

    # Host pre-tiles the operands so every DMA lands contiguous:
    #   xt{s} [ko, ki, c]     = X_e^T tiles, one tensor per c-subblock
    #   w1 [fb, ki, ko*128+f] = W1_e^T tiles
    #   w2 [do, fi, fo*128+d] = W2_e^T tiles
    #   yt [do, ki, c]        = out^T tiles          (f32)
    xt_ds = [
        nc.dram_tensor(f"xt{s}", [KO, P, c1 - c0], dt_x,
                       kind="ExternalInput").ap()
        for s, (c0, c1) in enumerate(CSUBS)
    ]
    w1_d = nc.dram_tensor("w1", [FO, P, KO * P], dt_w1, kind="ExternalInput").ap()
    w2_d = nc.dram_tensor("w2", [KO, P, FO * P], bf16, kind="ExternalInput").ap()
    yt_d = nc.dram_tensor("yt", [KO, P, CAP], f32, kind="ExternalOutput").ap()

    with _FastTC(nc) as tc, ExitStack() as ctx:
        xpool = ctx.enter_context(tc.tile_pool(name="xpool", bufs=1))
        w1pool = ctx.enter_context(tc.tile_pool(name="w1pool", bufs=4))
        w2pool = ctx.enter_context(tc.tile_pool(name="w2pool", bufs=3))
        hpool = ctx.enter_context(tc.tile_pool(name="hpool", bufs=1))
        opool = ctx.enter_context(tc.tile_pool(name="opool", bufs=4))
        cpool = ctx.enter_context(tc.tile_pool(name="cpool", bufs=1))
        # PSUM budget: 8 banks of (128 x 2KB). One pool shared by both
        # phases (and the HAM warmup), one tag per c-subblock, 3/3/2 slots
        # -> 8 banks total.
        pspool = ctx.enter_context(tc.tile_pool(name="ps", bufs=1, space="PSUM"))
        PS_BUFS = [3, 3, 2]

        bias = cpool.tile([P, 1], f32, name="bias")
        nc.any.memset(bias[:], 0.0)

        # HAM warmup: the PE clock-gate only opens (1.2 -> 2.4 GHz) after
        # ~3.4us of sustained matmul activity, and the DMA-paced first real
        # matmuls are too sparse to trip it (measured: warm at 28us without
        # this). Burn the input-DMA wait on dummy matmuls over a zeroed tile
        # so the real stream starts at full clock.
        warm = cpool.tile([P, 512], bf16, name="warm")
        nc.any.memset(warm[:], 0.0)
        wps = pspool.tile([P, 512], f32, name="ps_2", bufs=2)
        for _ in range(8):
            nc.tensor.matmul(wps[:], warm[:, :P], warm[:], start=True,
                             stop=True)

        # Startup order matters: the PE can start fb=0/s=0 after just
        # w1[0] + the s=0 third of xt. Interleave the first w1 prefetches
        # with the three contiguous xt slabs so neither stream starves.
        w1_pref = {}
        xts = [xpool.tile([P, CAP], dt_x, name=f"xt{ko}") for ko in range(KO)]
        for s, (c0, c1) in enumerate(CSUBS):
            if s < w1pool.bufs:
                t = w1pool.tile([P, KO * P], dt_w1, name="w1t")
                if s == 0:
                    # head first: fb=0's first chain needs only w1[0][ko=0]
                    # and xt[0][s=0]
                    nc.sync.dma_start(t[:, :P], w1_d[0][:, :P])
                    nc.sync.dma_start(xts[0][:, c0:c1], xt_ds[0][0])
                    nc.sync.dma_start(t[:, P:], w1_d[0][:, P:])
                else:
                    nc.sync.dma_start(t[:], w1_d[s])
                w1_pref[s] = t
            for ko in range(KO):
                if s == 0 and ko == 0:
                    continue
                nc.sync.dma_start(xts[ko][:, c0:c1], xt_ds[s][ko])
        t = w1pool.tile([P, KO * P], dt_w1, name="w1t")
        nc.sync.dma_start(t[:], w1_d[len(CSUBS)])
        w1_pref[len(CSUBS)] = t

        # h holds relu(W1 X^T) in f-major layout: 32 f-tiles side by side,
        # each (128, CAP), bf16 -> 80 KB/partition.
        h = hpool.tile([P, FO * CAP], bf16, name="h")

        # ---- phase 1: h[f, c] = relu(sum_d W1t[d, f] * Xt[d, c]) ----
        for fb in range(FO):
            if fb in w1_pref:
                w1t = w1_pref.pop(fb)
            else:
                w1t = w1pool.tile([P, KO * P], dt_w1, name="w1t")
                nc.sync.dma_start(w1t[:], w1_d[fb])
            for s, (c0, c1) in enumerate(CSUBS):
                ps = pspool.tile([P, c1 - c0], f32, name=f"ps_{s}",
                                 bufs=PS_BUFS[s])
                for ko in range(KO):
                    nc.tensor.matmul(
                        ps[:],
                        w1t[:, ko * P:(ko + 1) * P],
                        xts[ko][:, c0:c1],
                        start=(ko == 0),
                        stop=(ko == KO - 1),
                    )
                nc.scalar.activation(
                    h[:, fb * CAP + c0:fb * CAP + c1], ps[:], relu,
                    bias=bias[:],
                )

        # ---- phase 2: y[d, c] = sum_f W2t[f, d] * h[f, c], bf16 ----
        for do in range(KO):
            w2t = w2pool.tile([P, FO * P], bf16, name="w2t")
            nc.sync.dma_start(w2t[:], w2_d[do])
            for s, (c0, c1) in enumerate(CSUBS):
                ps = pspool.tile([P, c1 - c0], f32, name=f"ps_{s}",
                                 bufs=PS_BUFS[s])
                for fo in range(FO):
                    nc.tensor.matmul(
                        ps[:],
                        w2t[:, fo * P:(fo + 1) * P],
                        h[:, fo * CAP + c0:fo * CAP + c1],
                        start=(fo == 0),
                        stop=(fo == FO - 1),
                    )
                ot = opool.tile([P, c1 - c0], f32, name=f"ot{s}")
                nc.vector.tensor_copy(out=ot[:], in_=ps[:])
                nc.sync.dma_start(yt_d[do][:, c0:c1], ot[:])

    nc.compile()
    return nc


def _get_session():
    global _SESSION
    if _SESSION is None:
        _SESSION = _build_ffn_bass(VARIANT)
    return _SESSION


def _route(x_flat: np.ndarray, Wg: np.ndarray):
    """Mirror of the reference gating: top-2, softmax, capacity dropping in
    flat slot order, renormalize. Returns per-slot expert, position, weight."""
    logits = x_flat @ Wg.T.astype(np.float32)                 # (N, E) f32
    order = np.argsort(-logits, axis=1, kind="stable")[:, :TOPK]
    scores = np.take_along_axis(logits, order, axis=1)        # (N, k)
    m = scores.max(axis=1, keepdims=True)
    ex = np.exp(scores - m, dtype=np.float32)
    probs = ex / ex.sum(axis=1, keepdims=True)

    flat_idx = order.reshape(-1)                              # (NK,)
    onehot = flat_idx[:, None] == np.arange(E)[None, :]
    position = np.cumsum(onehot, axis=0)[np.arange(NK), flat_idx] - 1
    keep = position < CAP

    pk = probs.reshape(-1) * keep
    pk = pk.reshape(N, TOPK)
    pk = pk / (pk.sum(axis=1, keepdims=True) + np.float32(1e-9))
    flat_w = pk.reshape(-1).astype(np.float32)
    valid = flat_w > 0
    return flat_idx, position.astype(np.int64), flat_w, valid


def kernel(x, Wg, W1, W2, k):
    global LAST_RESULTS
    import ml_dtypes

    from concourse import bass_utils

    assert int(k) == TOPK
    x = np.asarray(x, np.float32)
    Wg = np.asarray(Wg, np.float32)
    W1 = np.asarray(W1, np.float32)
    W2 = np.asarray(W2, np.float32)

    x_flat = x.reshape(N, D)
    flat_idx, position, flat_w, valid = _route(x_flat, Wg)
    token_ids = np.repeat(np.arange(N), TOPK)

    # Dispatch: X_all[e, pos] = x[token] * w for each valid slot (positions
    # are unique per expert by construction, so plain assignment works).
    X_all = np.zeros((E, CAP, D), np.float32)
    v = np.nonzero(valid)[0]
    X_all[flat_idx[v], position[v]] = x_flat[token_ids[v]] * flat_w[v, None]

    nc = _get_session()

    in_maps = []
    for e in range(E):
        xt = np.ascontiguousarray(X_all[e].T).reshape(KO, P, CAP)
        w1 = (
            W1[e].T.reshape(KO, P, FO, P).transpose(2, 1, 0, 3)
            .reshape(FO, P, KO * P)
        )
        w2 = (
            W2[e].T.reshape(FO, P, KO, P).transpose(2, 1, 0, 3)
            .reshape(KO, P, FO * P)
        )
        w1_np = np.ascontiguousarray(w1)
        if VARIANT in ("bf16", "bf16w"):
            w1_np = w1_np.astype(ml_dtypes.bfloat16)
        m = {
            "w1": w1_np,
            "w2": np.ascontiguousarray(w2).astype(ml_dtypes.bfloat16),
        }
        for s, (c0, c1) in enumerate(CSUBS):
            xs = np.ascontiguousarray(xt[:, :, c0:c1])
            if VARIANT == "bf16":
                xs = xs.astype(ml_dtypes.bfloat16)
            m[f"xt{s}"] = xs
        in_maps.append(m)

    try:
        res = bass_utils.run_bass_kernel_spmd(
            nc, in_maps, core_ids=list(range(E))
        )
    except ModuleNotFoundError:
        # BASS_TRACE was set but this image lacks the axon NTFF hook module;
        # rerun without tracing.
        os.environ["BASS_NEVER_TRACE"] = "1"
        res = bass_utils.run_bass_kernel_spmd(
            nc, in_maps, core_ids=list(range(E))
        )
    LAST_RESULTS = res

    out_all = np.empty((E, CAP, D), np.float32)
    for e in range(E):
        out_all[e] = res.results[e]["yt"].reshape(D, CAP).T

    # Combine: each token sums its (up to k) slot outputs; weight was already
    # applied pre-dispatch.
    y_flat = np.zeros((N, D), np.float32)
    pos2 = position.reshape(N, TOPK)
    idx2 = flat_idx.reshape(N, TOPK)
    val2 = valid.reshape(N, TOPK)
    for j in range(TOPK):
        contrib = out_all[idx2[:, j], np.clip(pos2[:, j], 0, CAP - 1)]
        y_flat += contrib * val2[:, j, None].astype(np.float32)
    y = y_flat.reshape(B, S, D)

    # Load-balance aux loss.
    token_ctr = np.bincount(
        flat_idx, weights=valid.astype(np.float64), minlength=E
    ).astype(np.float32)
    imp_ctr = np.bincount(
        flat_idx, weights=flat_w.astype(np.float64), minlength=E
    ).astype(np.float32)
    aux = np.float32(
        np.sum((token_ctr / token_ctr.sum()) * (imp_ctr / imp_ctr.sum())) * E
    )
    return y, np.asarray(aux, np.float32)
